# revision 6
# baseline (speedup 1.0000x reference)
"""DLinear layer (nn_DLinearLayer) TRN2 Bass kernel.

Math (reference):
    trend[b,t,f]  = avgpool2(x)[b,t,f] = 0.5*(x[t]+x[t+1]), last: x[T-1]
    resid         = x - trend
    out[b,n,f]    = trend[:,:,f] @ trend_W[f] + trend_b[f,n]
                  + resid[:,:,f] @ residual_W[f] + residual_b[f,n]

Kernel identity used on device (with A = xT[t], B = xT[t+1], B[T-1]=xT[T-1]):
    S = A + B,  D = A - B
    out = 0.5*(S @ Wt + D @ Wr + ones x 2*(tb+rb))

Sharding: feature-expert — core k owns features {2k, 2k+1} (each feature's
[B,T] x [T,N] GEMM is independent; every weight byte is read exactly once
across the system). Host prep is layout-only: x is re-laid-out to [F,T,B]
so the contraction dim (t) lands on SBUF partitions.

Matmuls run in float32r (fp32 bits, relaxed PE mode: 1 cycle/row at
moving-dim >= 256 vs 4 cycles/row for strict fp32; measured rel-l2 error
~1.5e-4 on K=1024 dots).
"""

import numpy as np

import concourse.bass as bass
import concourse.mybir as mybir
import concourse.tile as tile
from concourse.bass_utils import run_bass_kernel_spmd

F, B, T, N = 16, 256, 1024, 1024
NCORES = 8
FL = F // NCORES          # features per core
TC = T // 128             # t chunks
NB = B // 128             # batch tiles (output partition tiles)
NH = N // 512             # output free-dim halves
F32 = mybir.dt.float32
F32R = mybir.dt.float32r


def _split_multi_waits(nc):
    """This container's walrus build accepts at most ONE sem wait per
    instruction ("Too many sync wait commands" in CoreV3Gen setupSyncWait).
    Tile emits 2+. Move excess waits onto nofuse NoOps placed immediately
    before the owning instruction on the same engine: engines execute their
    stream in order, so semantics are unchanged."""
    for fn in nc.m.functions:
        for blk in fn.blocks:
            out = []
            for inst in blk.instructions:
                si = inst.sync_info
                if si is not None and si.on_wait and len(si.on_wait) > 1:
                    waits = list(si.on_wait)
                    for j, w in enumerate(waits[:-1]):
                        out.append(mybir.InstNoOp(
                            name=f"{inst.name}-ws{j}",
                            engine=inst.engine,
                            bass_nofuse=True,
                            sync_info=mybir.SyncInfo(on_wait=[w], on_update=[]),
                        ))
                    si.on_wait = [waits[-1]]
                out.append(inst)
            blk.instructions[:] = out


def _build():
    nc = bass.Bass(trn_type="TRN2")

    xT_d = nc.dram_tensor("xT", [FL, T, B], F32R, kind="ExternalInput")
    wt_d = nc.dram_tensor("Wt", [FL, T, N], F32R, kind="ExternalInput")
    wr_d = nc.dram_tensor("Wr", [FL, T, N], F32R, kind="ExternalInput")
    tb_d = nc.dram_tensor("tb", [FL, N], F32R, kind="ExternalInput")
    rb_d = nc.dram_tensor("rb", [FL, N], F32R, kind="ExternalInput")
    ones_d = nc.dram_tensor("ones", [1, 128], F32R, kind="ExternalInput")
    out_d = nc.dram_tensor("out", [FL, B, N], F32, kind="ExternalOutput")

    with tile.TileContext(nc) as tc:
        with (
            tc.tile_pool(name="wp", bufs=24) as wp,
            tc.tile_pool(name="ab", bufs=20) as abp,
            tc.tile_pool(name="sd", bufs=20) as sdp,
            tc.tile_pool(name="bias", bufs=4) as biasp,
            tc.tile_pool(name="obuf", bufs=6) as obp,
            tc.tile_pool(name="const", bufs=1) as cp,
            tc.tile_pool(name="ps", bufs=4, space="PSUM") as psp,
        ):
            ones = cp.tile([1, 128], F32R)
            nc.sync.dma_start(ones[:], ones_d[:])

            for f in range(FL):
                # ---- weights for this feature: 8 chunks x [128, 1024] each
                wt_c, wr_c = [], []
                for c in range(TC):
                    w1 = wp.tile([128, N], F32R, tag="w")
                    nc.sync.dma_start(w1[:], wt_d[f, c * 128:(c + 1) * 128, :])
                    wt_c.append(w1)
                    w2 = wp.tile([128, N], F32R, tag="w")
                    nc.sync.dma_start(w2[:], wr_d[f, c * 128:(c + 1) * 128, :])
                    wr_c.append(w2)

                # ---- bias row: bias2 = 2*(tb+rb), [1, N]
                tbt = biasp.tile([1, N], F32R, tag="b")
                nc.sync.dma_start(tbt[:], tb_d[f:f + 1, :])
                rbt = biasp.tile([1, N], F32R, tag="b")
                nc.sync.dma_start(rbt[:], rb_d[f:f + 1, :])
                bias2 = biasp.tile([1, N], F32R, tag="b")
                nc.vector.tensor_add(bias2[:], tbt[:], rbt[:])
                nc.vector.tensor_scalar_mul(bias2[:], bias2[:], 2.0)

                # ---- activations: A = xT[t], Bt = xT[t+1] (last row dup)
                s_c, d_c = [], []
                for c in range(TC):
                    a = abp.tile([128, B], F32R, tag="ab")
                    nc.sync.dma_start(a[:], xT_d[f, c * 128:(c + 1) * 128, :])
                    bt = abp.tile([128, B], F32R, tag="ab")
                    if c < TC - 1:
                        nc.sync.dma_start(bt[:], xT_d[f, c * 128 + 1:(c + 1) * 128 + 1, :])
                    else:
                        nc.sync.dma_start(bt[:127, :], xT_d[f, c * 128 + 1:T, :])
                        nc.sync.dma_start(bt[127:128, :], xT_d[f, T - 1:T, :])
                    s = sdp.tile([128, B], F32R, tag="sd")
                    nc.vector.tensor_add(s[:], a[:], bt[:])
                    s_c.append(s)
                    d = sdp.tile([128, B], F32R, tag="sd")
                    nc.vector.tensor_sub(d[:], a[:], bt[:])
                    d_c.append(d)

                # ---- GEMMs: psum[b,n] = S@Wt + D@Wr + ones x bias2
                for b in range(NB):
                    bs = slice(b * 128, (b + 1) * 128)
                    psums = [psp.tile([128, 512], F32, tag="ps", name=f"ps_{f}_{b}_{h}")
                             for h in range(NH)]
                    for c in range(TC):
                        for h in range(NH):
                            ns = slice(h * 512, (h + 1) * 512)
                            nc.tensor.matmul(
                                psums[h][:], s_c[c][:, bs],
                                wt_c[c][:, ns],
                                start=(c == 0), stop=False)
                            nc.tensor.matmul(
                                psums[h][:], d_c[c][:, bs],
                                wr_c[c][:, ns],
                                start=False, stop=False)
                    for h in range(NH):
                        ns = slice(h * 512, (h + 1) * 512)
                        nc.tensor.matmul(
                            psums[h][:], ones[:],
                            bias2[:, ns],
                            start=False, stop=True)
                        ot = obp.tile([128, 512], F32, tag="o")
                        nc.scalar.mul(ot[:], psums[h][:], 0.5)
                        nc.sync.dma_start(out_d[f, bs, ns], ot[:])

    _split_multi_waits(nc)
    return nc


_NC_CACHE = []


def kernel(**inputs) -> np.ndarray:
    x = np.asarray(inputs["history_in"], dtype=np.float32)     # [B, T, F]
    wt = np.asarray(inputs["trend_W"], dtype=np.float32)       # [F, T, N]
    wr = np.asarray(inputs["residual_W"], dtype=np.float32)    # [F, T, N]
    tb = np.asarray(inputs["trend_b"], dtype=np.float32)       # [F, N]
    rb = np.asarray(inputs["residual_b"], dtype=np.float32)    # [F, N]

    xT = np.ascontiguousarray(x.transpose(2, 1, 0))            # [F, T, B]

    if not _NC_CACHE:
        _NC_CACHE.append(_build())
    nc = _NC_CACHE[0]

    in_maps = []
    for k in range(NCORES):
        sl = slice(FL * k, FL * (k + 1))
        in_maps.append({
            "xT": np.ascontiguousarray(xT[sl]),
            "Wt": np.ascontiguousarray(wt[sl]),
            "Wr": np.ascontiguousarray(wr[sl]),
            "tb": np.ascontiguousarray(tb[sl]),
            "rb": np.ascontiguousarray(rb[sl]),
            "ones": np.ones((1, 128), dtype=np.float32),
        })

    res = run_bass_kernel_spmd(nc, in_maps, core_ids=list(range(NCORES)))
    full = np.concatenate([r["out"] for r in res.results], axis=0)  # [F, B, N]
    return np.ascontiguousarray(full.transpose(1, 2, 0))            # [B, N, F]


# revision 7
# speedup vs baseline: 1.1961x; 1.1961x over previous
"""DLinear layer (nn_DLinearLayer) TRN2 Bass kernel.

Math (reference):
    trend[b,t,f]  = avgpool2(x)[b,t,f] = 0.5*(x[t]+x[t+1]), last: x[T-1]
    resid         = x - trend
    out[b,n,f]    = trend[:,:,f] @ trend_W[f] + trend_b[f,n]
                  + resid[:,:,f] @ residual_W[f] + residual_b[f,n]

Kernel identity used on device (with A = xT[t], B = xT[t+1], B[T-1]=xT[T-1]):
    S = A + B,  D = A - B
    out = 0.5*(S @ Wt + D @ Wr + ones x 2*(tb+rb))

Sharding: feature-expert — core k owns features {2k, 2k+1} (each feature's
[B,T] x [T,N] GEMM is independent; every weight byte is read exactly once
across the system). Host prep is layout-only: x is re-laid-out to [F,T,B]
so the contraction dim (t) lands on SBUF partitions.

Matmuls run in float32r (fp32 bits, relaxed PE mode: 1 cycle/row at
moving-dim >= 256 vs 4 cycles/row for strict fp32; measured rel-l2 error
~1.5e-4 on K=1024 dots).
"""

import numpy as np

import concourse.bass as bass
import concourse.mybir as mybir
import concourse.tile as tile
from concourse.bass_utils import run_bass_kernel_spmd

F, B, T, N = 16, 256, 1024, 1024
NCORES = 8
FL = F // NCORES          # features per core
TC = T // 128             # t chunks
NB = B // 128             # batch tiles (output partition tiles)
NH = N // 512             # output free-dim halves
F32 = mybir.dt.float32
F32R = mybir.dt.float32r


def _split_multi_waits(nc):
    """This container's walrus build accepts at most ONE sem wait per
    instruction ("Too many sync wait commands" in CoreV3Gen setupSyncWait).
    Tile emits 2+. Move excess waits onto nofuse NoOps placed immediately
    before the owning instruction on the same engine: engines execute their
    stream in order, so semantics are unchanged."""
    for fn in nc.m.functions:
        for blk in fn.blocks:
            out = []
            for inst in blk.instructions:
                si = inst.sync_info
                if si is not None and si.on_wait and len(si.on_wait) > 1:
                    waits = list(si.on_wait)
                    for j, w in enumerate(waits[:-1]):
                        out.append(mybir.InstNoOp(
                            name=f"{inst.name}-ws{j}",
                            engine=inst.engine,
                            bass_nofuse=True,
                            sync_info=mybir.SyncInfo(on_wait=[w], on_update=[]),
                        ))
                    si.on_wait = [waits[-1]]
                out.append(inst)
            blk.instructions[:] = out


def _build():
    nc = bass.Bass(trn_type="TRN2")

    xT_d = nc.dram_tensor("xT", [FL, T, B], F32R, kind="ExternalInput")
    wt_d = nc.dram_tensor("Wt", [FL, T, N], F32R, kind="ExternalInput")
    wr_d = nc.dram_tensor("Wr", [FL, T, N], F32R, kind="ExternalInput")
    tb_d = nc.dram_tensor("tb", [FL, N], F32R, kind="ExternalInput")
    rb_d = nc.dram_tensor("rb", [FL, N], F32R, kind="ExternalInput")
    ones_d = nc.dram_tensor("ones", [1, 128], F32R, kind="ExternalInput")
    out_d = nc.dram_tensor("out", [FL, B, N], F32, kind="ExternalOutput")

    with tile.TileContext(nc) as tc:
        with (
            tc.tile_pool(name="wp", bufs=32) as wp,
            tc.tile_pool(name="ab", bufs=10) as abp,
            tc.tile_pool(name="sd", bufs=34) as sdp,
            tc.tile_pool(name="bias", bufs=3) as biasp,
            tc.tile_pool(name="obuf", bufs=4) as obp,
            tc.tile_pool(name="const", bufs=1) as cp,
            tc.tile_pool(name="ps", bufs=4, space="PSUM") as psp,
        ):
            ones = cp.tile([1, 128], F32R)
            nc.sync.dma_start(ones[:], ones_d[:])

            # HWDGE can issue from both SP ("sync") and ACT ("scalar");
            # alternate big loads across them to double issue throughput.
            hwdge = [nc.sync, nc.scalar]

            wt_c, wr_c, s_c, d_c, bias2s = {}, {}, {}, {}, {}
            for f in range(FL):
                # ---- weights: 8 chunks x [128, 1024] per matrix
                for c in range(TC):
                    w1 = wp.tile([128, N], F32R, tag="w", name=f"wt_{f}_{c}")
                    hwdge[c % 2].dma_start(w1[:], wt_d[f, c * 128:(c + 1) * 128, :])
                    wt_c[f, c] = w1
                    w2 = wp.tile([128, N], F32R, tag="w", name=f"wr_{f}_{c}")
                    hwdge[(c + 1) % 2].dma_start(w2[:], wr_d[f, c * 128:(c + 1) * 128, :])
                    wr_c[f, c] = w2

                # ---- bias row: bias2 = 2*(tb+rb), [1, N]
                tbt = biasp.tile([1, N], F32R, tag="b", name=f"tb_{f}")
                nc.gpsimd.dma_start(tbt[:], tb_d[f:f + 1, :])
                rbt = biasp.tile([1, N], F32R, tag="b", name=f"rb_{f}")
                nc.gpsimd.dma_start(rbt[:], rb_d[f:f + 1, :])
                bias2 = biasp.tile([1, N], F32R, tag="b", name=f"b2_{f}")
                nc.vector.tensor_add(bias2[:], tbt[:], rbt[:])
                nc.vector.tensor_scalar_mul(bias2[:], bias2[:], 2.0)
                bias2s[f] = bias2

                # ---- activations: A = xT[t], Bt = xT[t+1] (last row dup)
                for c in range(TC):
                    a = abp.tile([128, B], F32R, tag="ab", name=f"a_{f}_{c}")
                    hwdge[c % 2].dma_start(a[:], xT_d[f, c * 128:(c + 1) * 128, :])
                    bt = abp.tile([128, B], F32R, tag="ab", name=f"bt_{f}_{c}")
                    if c < TC - 1:
                        hwdge[(c + 1) % 2].dma_start(bt[:], xT_d[f, c * 128 + 1:(c + 1) * 128 + 1, :])
                    else:
                        hwdge[(c + 1) % 2].dma_start(bt[:127, :], xT_d[f, c * 128 + 1:T, :])
                        nc.gpsimd.dma_start(bt[127:128, :], xT_d[f, T - 1:T, :])
                    s = sdp.tile([128, B], F32R, tag="sd", name=f"s_{f}_{c}")
                    nc.vector.tensor_add(s[:], a[:], bt[:])
                    s_c[f, c] = s
                    d = sdp.tile([128, B], F32R, tag="sd", name=f"d_{f}_{c}")
                    nc.vector.tensor_sub(d[:], a[:], bt[:])
                    d_c[f, c] = d

            # ---- GEMMs: psum[b,n] = S@Wt + D@Wr + ones x bias2
            for f in range(FL):
                for b in range(NB):
                    bs = slice(b * 128, (b + 1) * 128)
                    psums = [psp.tile([128, 512], F32, tag="ps", name=f"ps_{f}_{b}_{h}")
                             for h in range(NH)]
                    for c in range(TC):
                        for h in range(NH):
                            ns = slice(h * 512, (h + 1) * 512)
                            nc.tensor.matmul(
                                psums[h][:], s_c[f, c][:, bs],
                                wt_c[f, c][:, ns],
                                start=(c == 0), stop=False)
                            nc.tensor.matmul(
                                psums[h][:], d_c[f, c][:, bs],
                                wr_c[f, c][:, ns],
                                start=False, stop=False)
                    for h in range(NH):
                        ns = slice(h * 512, (h + 1) * 512)
                        nc.tensor.matmul(
                            psums[h][:], ones[:],
                            bias2s[f][:, ns],
                            start=False, stop=True)
                        ot = obp.tile([128, 512], F32, tag="o", name=f"o_{f}_{b}_{h}")
                        nc.scalar.mul(ot[:], psums[h][:], 0.5)
                        nc.sync.dma_start(out_d[f, bs, ns], ot[:])

    _split_multi_waits(nc)
    return nc


_NC_CACHE = []


def kernel(**inputs) -> np.ndarray:
    x = np.asarray(inputs["history_in"], dtype=np.float32)     # [B, T, F]
    wt = np.asarray(inputs["trend_W"], dtype=np.float32)       # [F, T, N]
    wr = np.asarray(inputs["residual_W"], dtype=np.float32)    # [F, T, N]
    tb = np.asarray(inputs["trend_b"], dtype=np.float32)       # [F, N]
    rb = np.asarray(inputs["residual_b"], dtype=np.float32)    # [F, N]

    xT = np.ascontiguousarray(x.transpose(2, 1, 0))            # [F, T, B]

    if not _NC_CACHE:
        _NC_CACHE.append(_build())
    nc = _NC_CACHE[0]

    in_maps = []
    for k in range(NCORES):
        sl = slice(FL * k, FL * (k + 1))
        in_maps.append({
            "xT": np.ascontiguousarray(xT[sl]),
            "Wt": np.ascontiguousarray(wt[sl]),
            "Wr": np.ascontiguousarray(wr[sl]),
            "tb": np.ascontiguousarray(tb[sl]),
            "rb": np.ascontiguousarray(rb[sl]),
            "ones": np.ones((1, 128), dtype=np.float32),
        })

    res = run_bass_kernel_spmd(nc, in_maps, core_ids=list(range(NCORES)))
    full = np.concatenate([r["out"] for r in res.results], axis=0)  # [F, B, N]
    return np.ascontiguousarray(full.transpose(1, 2, 0))            # [B, N, F]


# revision 8
# speedup vs baseline: 1.2861x; 1.0753x over previous
"""DLinear layer (nn_DLinearLayer) TRN2 Bass kernel.

Math (reference):
    trend[b,t,f]  = avgpool2(x)[b,t,f] = 0.5*(x[t]+x[t+1]), last: x[T-1]
    resid         = x - trend
    out[b,n,f]    = trend[:,:,f] @ trend_W[f] + trend_b[f,n]
                  + resid[:,:,f] @ residual_W[f] + residual_b[f,n]

Kernel identity used on device (with A = xT[t], B = xT[t+1], B[T-1]=xT[T-1]):
    S = A + B,  D = A - B
    out = 0.5*(S @ Wt + D @ Wr + ones x 2*(tb+rb))

Sharding: feature-expert — core k owns features {2k, 2k+1} (each feature's
[B,T] x [T,N] GEMM is independent; every weight byte is read exactly once
across the system). Host prep is layout-only: x is re-laid-out to [F,T,B]
so the contraction dim (t) lands on SBUF partitions.

Matmuls run in float32r (fp32 bits, relaxed PE mode: 1 cycle/row at
moving-dim >= 256 vs 4 cycles/row for strict fp32; measured rel-l2 error
~1.5e-4 on K=1024 dots).
"""

import numpy as np

import concourse.bass as bass
import concourse.mybir as mybir
import concourse.tile as tile
from concourse.bass_utils import run_bass_kernel_spmd

F, B, T, N = 16, 256, 1024, 1024
NCORES = 8
FL = F // NCORES          # features per core
TC = T // 128             # t chunks
NB = B // 128             # batch tiles (output partition tiles)
NH = N // 512             # output free-dim halves
F32 = mybir.dt.float32
F32R = mybir.dt.float32r


def _split_multi_waits(nc):
    """This container's walrus build accepts at most ONE sem wait per
    instruction ("Too many sync wait commands" in CoreV3Gen setupSyncWait).
    Tile emits 2+. Move excess waits onto nofuse NoOps placed immediately
    before the owning instruction on the same engine: engines execute their
    stream in order, so semantics are unchanged."""
    for fn in nc.m.functions:
        for blk in fn.blocks:
            out = []
            for inst in blk.instructions:
                si = inst.sync_info
                if si is not None and si.on_wait and len(si.on_wait) > 1:
                    waits = list(si.on_wait)
                    for j, w in enumerate(waits[:-1]):
                        out.append(mybir.InstNoOp(
                            name=f"{inst.name}-ws{j}",
                            engine=inst.engine,
                            bass_nofuse=True,
                            sync_info=mybir.SyncInfo(on_wait=[w], on_update=[]),
                        ))
                    si.on_wait = [waits[-1]]
                out.append(inst)
            blk.instructions[:] = out


def _build():
    nc = bass.Bass(trn_type="TRN2")

    xT_d = nc.dram_tensor("xT", [FL, T, B], F32R, kind="ExternalInput")
    wt_d = nc.dram_tensor("Wt", [FL, T, N], F32R, kind="ExternalInput")
    wr_d = nc.dram_tensor("Wr", [FL, T, N], F32R, kind="ExternalInput")
    tb_d = nc.dram_tensor("tb", [FL, N], F32R, kind="ExternalInput")
    rb_d = nc.dram_tensor("rb", [FL, N], F32R, kind="ExternalInput")
    ones_d = nc.dram_tensor("ones", [1, 128], F32R, kind="ExternalInput")
    out_d = nc.dram_tensor("out", [FL, B, N], F32, kind="ExternalOutput")

    with tile.TileContext(nc) as tc:
        with (
            tc.tile_pool(name="wp", bufs=20) as wp,
            tc.tile_pool(name="ab", bufs=4) as abp,
            tc.tile_pool(name="sd", bufs=4) as sdp,
            tc.tile_pool(name="bias", bufs=3) as biasp,
            tc.tile_pool(name="obuf", bufs=4) as obp,
            tc.tile_pool(name="const", bufs=1) as cp,
            tc.tile_pool(name="ps", bufs=8, space="PSUM") as psp,
        ):
            ones = cp.tile([1, 128], F32R)
            nc.gpsimd.dma_start(ones[:], ones_d[:])

            # HWDGE can issue from both SP ("sync") and ACT ("scalar").
            hwdge = [nc.sync, nc.scalar]

            # ---- activations first (small; unblocks S/D while W streams in).
            # One batched DMA per tensor: per-DMA issue cost on the HWDGE
            # sequencer is ~0.7us regardless of size.
            a_all, b_all, s_all, d_all, bias2s = {}, {}, {}, {}, {}
            for f in range(FL):
                a = abp.tile([128, TC, B], F32R, tag="ab", name=f"a_{f}")
                hwdge[f % 2].dma_start(
                    a[:], xT_d[f].rearrange("(c p) b -> p c b", p=128))
                a_all[f] = a
                # B = xT[t+1], last row duplicated: chunks 0..6 in one DMA
                # (rows 1..897), chunk 7 rows 897..1023 + dup of row 1023.
                bt = abp.tile([128, TC, B], F32R, tag="ab", name=f"bt_{f}")
                hwdge[(f + 1) % 2].dma_start(
                    bt[:, 0:TC - 1, :],
                    xT_d[f][1:1 + (TC - 1) * 128, :].rearrange(
                        "(c p) b -> p c b", p=128))
                hwdge[f % 2].dma_start(
                    bt[0:127, TC - 1, :], xT_d[f][(TC - 1) * 128 + 1:T, :])
                nc.gpsimd.dma_start(
                    bt[127:128, TC - 1, :], xT_d[f][T - 1:T, :])
                b_all[f] = bt

                tbt = biasp.tile([1, N], F32R, tag="b", name=f"tb_{f}")
                nc.gpsimd.dma_start(tbt[:], tb_d[f:f + 1, :])
                rbt = biasp.tile([1, N], F32R, tag="b", name=f"rb_{f}")
                nc.gpsimd.dma_start(rbt[:], rb_d[f:f + 1, :])
                bias2 = biasp.tile([1, N], F32R, tag="b", name=f"b2_{f}")
                nc.vector.tensor_add(bias2[:], tbt[:], rbt[:])
                nc.vector.tensor_scalar_mul(bias2[:], bias2[:], 2.0)
                bias2s[f] = bias2

            # ---- S = A+B, D = A-B, one batched DVE op each
            for f in range(FL):
                s = sdp.tile([128, TC, B], F32R, tag="sd", name=f"s_{f}")
                nc.vector.tensor_add(s[:], a_all[f][:], b_all[f][:])
                s_all[f] = s
                d = sdp.tile([128, TC, B], F32R, tag="sd", name=f"d_{f}")
                nc.vector.tensor_sub(d[:], a_all[f][:], b_all[f][:])
                d_all[f] = d

            # ---- weights, issued in exact consumption order
            wt_c, wr_c = {}, {}
            for f in range(FL):
                for c in range(TC):
                    w1 = wp.tile([128, N], F32R, tag="w", name=f"wt_{f}_{c}")
                    hwdge[c % 2].dma_start(w1[:], wt_d[f, c * 128:(c + 1) * 128, :])
                    wt_c[f, c] = w1
                    w2 = wp.tile([128, N], F32R, tag="w", name=f"wr_{f}_{c}")
                    hwdge[(c + 1) % 2].dma_start(w2[:], wr_d[f, c * 128:(c + 1) * 128, :])
                    wr_c[f, c] = w2

            # ---- GEMMs: each W chunk fully consumed on arrival
            for f in range(FL):
                psums = {(b, h): psp.tile([128, 512], F32, tag="ps",
                                          name=f"ps_{f}_{b}_{h}")
                         for b in range(NB) for h in range(NH)}
                for c in range(TC):
                    for b in range(NB):
                        lhs_s = s_all[f][:, c, b * 128:(b + 1) * 128]
                        lhs_d = d_all[f][:, c, b * 128:(b + 1) * 128]
                        for h in range(NH):
                            ns = slice(h * 512, (h + 1) * 512)
                            nc.tensor.matmul(
                                psums[b, h][:], lhs_s, wt_c[f, c][:, ns],
                                start=(c == 0), stop=False)
                            nc.tensor.matmul(
                                psums[b, h][:], lhs_d, wr_c[f, c][:, ns],
                                start=False, stop=False)
                for b in range(NB):
                    bs = slice(b * 128, (b + 1) * 128)
                    for h in range(NH):
                        ns = slice(h * 512, (h + 1) * 512)
                        nc.tensor.matmul(
                            psums[b, h][:], ones[:], bias2s[f][:, ns],
                            start=False, stop=True)
                        ot = obp.tile([128, 512], F32, tag="o", name=f"o_{f}_{b}_{h}")
                        nc.scalar.mul(ot[:], psums[b, h][:], 0.5)
                        nc.gpsimd.dma_start(out_d[f, bs, ns], ot[:])

    _split_multi_waits(nc)
    return nc


_NC_CACHE = []


def kernel(**inputs) -> np.ndarray:
    x = np.asarray(inputs["history_in"], dtype=np.float32)     # [B, T, F]
    wt = np.asarray(inputs["trend_W"], dtype=np.float32)       # [F, T, N]
    wr = np.asarray(inputs["residual_W"], dtype=np.float32)    # [F, T, N]
    tb = np.asarray(inputs["trend_b"], dtype=np.float32)       # [F, N]
    rb = np.asarray(inputs["residual_b"], dtype=np.float32)    # [F, N]

    xT = np.ascontiguousarray(x.transpose(2, 1, 0))            # [F, T, B]

    if not _NC_CACHE:
        _NC_CACHE.append(_build())
    nc = _NC_CACHE[0]

    in_maps = []
    for k in range(NCORES):
        sl = slice(FL * k, FL * (k + 1))
        in_maps.append({
            "xT": np.ascontiguousarray(xT[sl]),
            "Wt": np.ascontiguousarray(wt[sl]),
            "Wr": np.ascontiguousarray(wr[sl]),
            "tb": np.ascontiguousarray(tb[sl]),
            "rb": np.ascontiguousarray(rb[sl]),
            "ones": np.ones((1, 128), dtype=np.float32),
        })

    res = run_bass_kernel_spmd(nc, in_maps, core_ids=list(range(NCORES)))
    full = np.concatenate([r["out"] for r in res.results], axis=0)  # [F, B, N]
    return np.ascontiguousarray(full.transpose(1, 2, 0))            # [B, N, F]


# revision 9
# speedup vs baseline: 1.5531x; 1.2076x over previous
"""DLinear layer (nn_DLinearLayer) TRN2 Bass kernel.

Math (reference):
    trend[b,t,f]  = avgpool2(x)[b,t,f] = 0.5*(x[t]+x[t+1]), last: x[T-1]
    resid         = x - trend
    out[b,n,f]    = trend[:,:,f] @ trend_W[f] + trend_b[f,n]
                  + resid[:,:,f] @ residual_W[f] + residual_b[f,n]

Kernel identity used on device (with A = xT[t], B = xT[t+1], B[T-1]=xT[T-1]):
    S = A + B,  D = A - B
    out = 0.5*(S @ Wt + D @ Wr + ones x 2*(tb+rb))

Sharding: feature-expert — core k owns features {2k, 2k+1} (each feature's
[B,T] x [T,N] GEMM is independent; every weight byte is read exactly once
across the system). Host prep is layout-only: x is re-laid-out to [F,T,B]
so the contraction dim (t) lands on SBUF partitions.

Matmuls run in float32r (fp32 bits, relaxed PE mode: 1 cycle/row at
moving-dim >= 256 vs 4 cycles/row for strict fp32; measured rel-l2 error
~1.5e-4 on K=1024 dots).
"""

import numpy as np

import concourse.bass as bass
import concourse.mybir as mybir
import concourse.tile as tile
from concourse.bass_utils import run_bass_kernel_spmd

F, B, T, N = 16, 256, 1024, 1024
NCORES = 8
FL = F // NCORES          # features per core
TC = T // 128             # t chunks
NB = B // 128             # batch tiles (output partition tiles)
NH = N // 512             # output free-dim halves
F32 = mybir.dt.float32
F32R = mybir.dt.float32r


def _split_multi_waits(nc):
    """This container's walrus build accepts at most ONE sem wait per
    instruction ("Too many sync wait commands" in CoreV3Gen setupSyncWait).
    Tile emits 2+. Move excess waits onto nofuse NoOps placed immediately
    before the owning instruction on the same engine: engines execute their
    stream in order, so semantics are unchanged."""
    for fn in nc.m.functions:
        for blk in fn.blocks:
            out = []
            for inst in blk.instructions:
                si = inst.sync_info
                if si is not None and si.on_wait and len(si.on_wait) > 1:
                    waits = list(si.on_wait)
                    for j, w in enumerate(waits[:-1]):
                        out.append(mybir.InstNoOp(
                            name=f"{inst.name}-ws{j}",
                            engine=inst.engine,
                            bass_nofuse=True,
                            sync_info=mybir.SyncInfo(on_wait=[w], on_update=[]),
                        ))
                    si.on_wait = [waits[-1]]
                out.append(inst)
            blk.instructions[:] = out


def _build():
    nc = bass.Bass(trn_type="TRN2")

    xA_d = nc.dram_tensor("xA", [FL, 128, TC, B], F32R, kind="ExternalInput")
    xB_d = nc.dram_tensor("xB", [FL, 128, TC, B], F32R, kind="ExternalInput")
    wt_d = nc.dram_tensor("Wt", [FL, T, N], F32R, kind="ExternalInput")
    wr_d = nc.dram_tensor("Wr", [FL, T, N], F32R, kind="ExternalInput")
    tb_d = nc.dram_tensor("tb", [FL, N], F32R, kind="ExternalInput")
    rb_d = nc.dram_tensor("rb", [FL, N], F32R, kind="ExternalInput")
    ones_d = nc.dram_tensor("ones", [1, 128], F32R, kind="ExternalInput")
    out_d = nc.dram_tensor("out", [FL, B, N], F32, kind="ExternalOutput")

    with tile.TileContext(nc) as tc:
        with (
            tc.tile_pool(name="wp", bufs=20) as wp,
            tc.tile_pool(name="ab", bufs=4) as abp,
            tc.tile_pool(name="sd", bufs=4) as sdp,
            tc.tile_pool(name="bias", bufs=3) as biasp,
            tc.tile_pool(name="obuf", bufs=4) as obp,
            tc.tile_pool(name="const", bufs=1) as cp,
            tc.tile_pool(name="ps", bufs=8, space="PSUM") as psp,
        ):
            ones = cp.tile([1, 128], F32R)
            nc.gpsimd.dma_start(ones[:], ones_d[:])

            # HWDGE issues from both SP ("sync") and ACT ("scalar").
            hwdge = [nc.sync, nc.scalar]

            # bias rows via SWDGE (separate path, tiny)
            bias2s = {}
            for f in range(FL):
                tbt = biasp.tile([1, N], F32R, tag="b", name=f"tb_{f}")
                nc.gpsimd.dma_start(tbt[:], tb_d[f:f + 1, :])
                rbt = biasp.tile([1, N], F32R, tag="b", name=f"rb_{f}")
                nc.gpsimd.dma_start(rbt[:], rb_d[f:f + 1, :])
                bias2 = biasp.tile([1, N], F32R, tag="b", name=f"b2_{f}")
                nc.vector.tensor_add(bias2[:], tbt[:], rbt[:])
                nc.vector.tensor_scalar_mul(bias2[:], bias2[:], 2.0)
                bias2s[f] = bias2

            # Per feature: x first (partition-major host layout -> one
            # 128x8KB-descriptor DMA per tensor), then weights in exact
            # consumption order so the HBM stream arrives chunk by chunk.
            a_all, b_all, s_all, d_all, wt_c, wr_c = {}, {}, {}, {}, {}, {}
            for f in range(FL):
                a = abp.tile([128, TC, B], F32R, tag="ab", name=f"a_{f}")
                hwdge[0].dma_start(a[:], xA_d[f])
                a_all[f] = a
                bt = abp.tile([128, TC, B], F32R, tag="ab", name=f"bt_{f}")
                hwdge[1].dma_start(bt[:], xB_d[f])
                b_all[f] = bt

                s = sdp.tile([128, TC, B], F32R, tag="sd", name=f"s_{f}")
                nc.vector.tensor_add(s[:], a[:], bt[:])
                s_all[f] = s
                d = sdp.tile([128, TC, B], F32R, tag="sd", name=f"d_{f}")
                nc.vector.tensor_sub(d[:], a[:], bt[:])
                d_all[f] = d

                for c in range(TC):
                    w1 = wp.tile([128, N], F32R, tag="w", name=f"wt_{f}_{c}")
                    hwdge[c % 2].dma_start(w1[:], wt_d[f, c * 128:(c + 1) * 128, :])
                    wt_c[f, c] = w1
                    w2 = wp.tile([128, N], F32R, tag="w", name=f"wr_{f}_{c}")
                    hwdge[(c + 1) % 2].dma_start(w2[:], wr_d[f, c * 128:(c + 1) * 128, :])
                    wr_c[f, c] = w2

            # ---- GEMMs: each W chunk fully consumed on arrival
            for f in range(FL):
                psums = {(b, h): psp.tile([128, 512], F32, tag="ps",
                                          name=f"ps_{f}_{b}_{h}")
                         for b in range(NB) for h in range(NH)}
                for c in range(TC):
                    for b in range(NB):
                        lhs_s = s_all[f][:, c, b * 128:(b + 1) * 128]
                        lhs_d = d_all[f][:, c, b * 128:(b + 1) * 128]
                        for h in range(NH):
                            ns = slice(h * 512, (h + 1) * 512)
                            nc.tensor.matmul(
                                psums[b, h][:], lhs_s, wt_c[f, c][:, ns],
                                start=(c == 0), stop=False)
                            nc.tensor.matmul(
                                psums[b, h][:], lhs_d, wr_c[f, c][:, ns],
                                start=False, stop=False)
                for b in range(NB):
                    bs = slice(b * 128, (b + 1) * 128)
                    for h in range(NH):
                        ns = slice(h * 512, (h + 1) * 512)
                        nc.tensor.matmul(
                            psums[b, h][:], ones[:], bias2s[f][:, ns],
                            start=False, stop=True)
                        ot = obp.tile([128, 512], F32, tag="o", name=f"o_{f}_{b}_{h}")
                        nc.scalar.mul(ot[:], psums[b, h][:], 0.5)
                        nc.gpsimd.dma_start(out_d[f, bs, ns], ot[:])

    _split_multi_waits(nc)
    return nc


_NC_CACHE = []


def kernel(**inputs) -> np.ndarray:
    x = np.asarray(inputs["history_in"], dtype=np.float32)     # [B, T, F]
    wt = np.asarray(inputs["trend_W"], dtype=np.float32)       # [F, T, N]
    wr = np.asarray(inputs["residual_W"], dtype=np.float32)    # [F, T, N]
    tb = np.asarray(inputs["trend_b"], dtype=np.float32)       # [F, N]
    rb = np.asarray(inputs["residual_b"], dtype=np.float32)    # [F, N]

    xT = x.transpose(2, 1, 0)                                  # [F, T, B] view
    # partition-major: xA[f, p, c, b] = xT[f, c*128+p, b]
    xA = np.ascontiguousarray(
        xT.reshape(F, TC, 128, B).transpose(0, 2, 1, 3))       # [F, 128, TC, B]
    # shifted-by-one-row copy with last row duplicated
    xTs = np.concatenate([xT[:, 1:, :], xT[:, T - 1:T, :]], axis=1)
    xB = np.ascontiguousarray(
        xTs.reshape(F, TC, 128, B).transpose(0, 2, 1, 3))      # [F, 128, TC, B]

    if not _NC_CACHE:
        _NC_CACHE.append(_build())
    nc = _NC_CACHE[0]

    in_maps = []
    for k in range(NCORES):
        sl = slice(FL * k, FL * (k + 1))
        in_maps.append({
            "xA": np.ascontiguousarray(xA[sl]),
            "xB": np.ascontiguousarray(xB[sl]),
            "Wt": np.ascontiguousarray(wt[sl]),
            "Wr": np.ascontiguousarray(wr[sl]),
            "tb": np.ascontiguousarray(tb[sl]),
            "rb": np.ascontiguousarray(rb[sl]),
            "ones": np.ones((1, 128), dtype=np.float32),
        })

    res = run_bass_kernel_spmd(nc, in_maps, core_ids=list(range(NCORES)))
    full = np.concatenate([r["out"] for r in res.results], axis=0)  # [F, B, N]
    return np.ascontiguousarray(full.transpose(1, 2, 0))            # [B, N, F]


# revision 10
# speedup vs baseline: 1.5696x; 1.0106x over previous
"""DLinear layer (nn_DLinearLayer) TRN2 Bass kernel.

Math (reference):
    trend[b,t,f]  = avgpool2(x)[b,t,f] = 0.5*(x[t]+x[t+1]), last: x[T-1]
    resid         = x - trend
    out[b,n,f]    = trend[:,:,f] @ trend_W[f] + trend_b[f,n]
                  + resid[:,:,f] @ residual_W[f] + residual_b[f,n]

Kernel identity used on device (with A = xT[t], B = xT[t+1], B[T-1]=xT[T-1]):
    S = A + B,  D = A - B
    out = 0.5*(S @ Wt + D @ Wr + ones x 2*(tb+rb))

Sharding: feature-expert — core k owns features {2k, 2k+1} (each feature's
[B,T] x [T,N] GEMM is independent; every weight byte is read exactly once
across the system). Host prep is layout-only: x is re-laid-out to [F,T,B]
so the contraction dim (t) lands on SBUF partitions.

Matmuls run in float32r (fp32 bits, relaxed PE mode: 1 cycle/row at
moving-dim >= 256 vs 4 cycles/row for strict fp32; measured rel-l2 error
~1.5e-4 on K=1024 dots).
"""

import numpy as np

import concourse.bass as bass
import concourse.mybir as mybir
import concourse.tile as tile
from concourse.bass_utils import run_bass_kernel_spmd

F, B, T, N = 16, 256, 1024, 1024
NCORES = 8
FL = F // NCORES          # features per core
TC = T // 128             # t chunks
NB = B // 128             # batch tiles (output partition tiles)
NH = N // 512             # output free-dim halves
F32 = mybir.dt.float32
F32R = mybir.dt.float32r


def _split_multi_waits(nc):
    """This container's walrus build accepts at most ONE sem wait per
    instruction ("Too many sync wait commands" in CoreV3Gen setupSyncWait).
    Tile emits 2+. Move excess waits onto nofuse NoOps placed immediately
    before the owning instruction on the same engine: engines execute their
    stream in order, so semantics are unchanged."""
    for fn in nc.m.functions:
        for blk in fn.blocks:
            out = []
            for inst in blk.instructions:
                si = inst.sync_info
                if si is not None and si.on_wait and len(si.on_wait) > 1:
                    waits = list(si.on_wait)
                    for j, w in enumerate(waits[:-1]):
                        out.append(mybir.InstNoOp(
                            name=f"{inst.name}-ws{j}",
                            engine=inst.engine,
                            bass_nofuse=True,
                            sync_info=mybir.SyncInfo(on_wait=[w], on_update=[]),
                        ))
                    si.on_wait = [waits[-1]]
                out.append(inst)
            blk.instructions[:] = out


def _build():
    nc = bass.Bass(trn_type="TRN2")

    xA_d = nc.dram_tensor("xA", [FL, 128, TC, B], F32R, kind="ExternalInput")
    xB_d = nc.dram_tensor("xB", [FL, 128, TC, B], F32R, kind="ExternalInput")
    wt_d = nc.dram_tensor("Wt", [FL, T, N], F32R, kind="ExternalInput")
    wr_d = nc.dram_tensor("Wr", [FL, T, N], F32R, kind="ExternalInput")
    tb_d = nc.dram_tensor("tb", [FL, N], F32R, kind="ExternalInput")
    rb_d = nc.dram_tensor("rb", [FL, N], F32R, kind="ExternalInput")
    ones_d = nc.dram_tensor("ones", [1, 128], F32R, kind="ExternalInput")
    out_d = nc.dram_tensor("out", [FL, B, N], F32, kind="ExternalOutput")

    with tile.TileContext(nc) as tc:
        with (
            tc.tile_pool(name="wp", bufs=20) as wp,
            tc.tile_pool(name="ab", bufs=4) as abp,
            tc.tile_pool(name="sd", bufs=4) as sdp,
            tc.tile_pool(name="bias", bufs=3) as biasp,
            tc.tile_pool(name="obuf", bufs=4) as obp,
            tc.tile_pool(name="const", bufs=1) as cp,
            tc.tile_pool(name="ps", bufs=8, space="PSUM") as psp,
        ):
            ones = cp.tile([1, 128], F32R)
            nc.gpsimd.dma_start(ones[:], ones_d[:])

            # HWDGE issues from both SP ("sync") and ACT ("scalar").
            hwdge = [nc.sync, nc.scalar]

            # bias rows via SWDGE (separate path, tiny)
            bias2s = {}
            for f in range(FL):
                tbt = biasp.tile([1, N], F32R, tag="b", name=f"tb_{f}")
                nc.gpsimd.dma_start(tbt[:], tb_d[f:f + 1, :])
                rbt = biasp.tile([1, N], F32R, tag="b", name=f"rb_{f}")
                nc.gpsimd.dma_start(rbt[:], rb_d[f:f + 1, :])
                bias2 = biasp.tile([1, N], F32R, tag="b", name=f"b2_{f}")
                nc.vector.tensor_add(bias2[:], tbt[:], rbt[:])
                nc.vector.tensor_scalar_mul(bias2[:], bias2[:], 2.0)
                bias2s[f] = bias2

            # Per feature: x halves interleaved with the first W chunks
            # so neither the S/D inputs nor W c0 arrive late. Partition-major
            # host layout -> 8KB contiguous per partition per x DMA.
            a_all, b_all, s_all, d_all, wt_c, wr_c = {}, {}, {}, {}, {}, {}
            HC = TC // 2
            for f in range(FL):
                a = abp.tile([128, TC, B], F32R, tag="ab", name=f"a_{f}")
                b = abp.tile([128, TC, B], F32R, tag="ab", name=f"bt_{f}")
                s = sdp.tile([128, TC, B], F32R, tag="sd", name=f"s_{f}")
                dd = sdp.tile([128, TC, B], F32R, tag="sd", name=f"d_{f}")
                a_all[f], b_all[f], s_all[f], d_all[f] = a, b, s, dd

                # first x half
                hwdge[0].dma_start(a[:, 0:HC, :], xA_d[f, :, 0:HC, :])
                hwdge[1].dma_start(b[:, 0:HC, :], xB_d[f, :, 0:HC, :])
                nc.vector.tensor_add(s[:, 0:HC, :], a[:, 0:HC, :], b[:, 0:HC, :])
                nc.vector.tensor_sub(dd[:, 0:HC, :], a[:, 0:HC, :], b[:, 0:HC, :])
                # first W chunk pair
                for c in range(1):
                    w1 = wp.tile([128, N], F32R, tag="w", name=f"wt_{f}_{c}")
                    hwdge[0].dma_start(w1[:], wt_d[f, c * 128:(c + 1) * 128, :])
                    wt_c[f, c] = w1
                    w2 = wp.tile([128, N], F32R, tag="w", name=f"wr_{f}_{c}")
                    hwdge[1].dma_start(w2[:], wr_d[f, c * 128:(c + 1) * 128, :])
                    wr_c[f, c] = w2
                # second x half
                hwdge[0].dma_start(a[:, HC:TC, :], xA_d[f, :, HC:TC, :])
                hwdge[1].dma_start(b[:, HC:TC, :], xB_d[f, :, HC:TC, :])
                nc.vector.tensor_add(s[:, HC:TC, :], a[:, HC:TC, :], b[:, HC:TC, :])
                nc.vector.tensor_sub(dd[:, HC:TC, :], a[:, HC:TC, :], b[:, HC:TC, :])
                # remaining W chunks in consumption order
                for c in range(1, TC):
                    w1 = wp.tile([128, N], F32R, tag="w", name=f"wt_{f}_{c}")
                    hwdge[c % 2].dma_start(w1[:], wt_d[f, c * 128:(c + 1) * 128, :])
                    wt_c[f, c] = w1
                    w2 = wp.tile([128, N], F32R, tag="w", name=f"wr_{f}_{c}")
                    hwdge[(c + 1) % 2].dma_start(w2[:], wr_d[f, c * 128:(c + 1) * 128, :])
                    wr_c[f, c] = w2

            # ---- GEMMs: each W chunk fully consumed on arrival
            for f in range(FL):
                psums = {(b, h): psp.tile([128, 512], F32, tag="ps",
                                          name=f"ps_{f}_{b}_{h}")
                         for b in range(NB) for h in range(NH)}
                for c in range(TC):
                    for b in range(NB):
                        lhs_s = s_all[f][:, c, b * 128:(b + 1) * 128]
                        lhs_d = d_all[f][:, c, b * 128:(b + 1) * 128]
                        for h in range(NH):
                            ns = slice(h * 512, (h + 1) * 512)
                            nc.tensor.matmul(
                                psums[b, h][:], lhs_s, wt_c[f, c][:, ns],
                                start=(c == 0), stop=False)
                            nc.tensor.matmul(
                                psums[b, h][:], lhs_d, wr_c[f, c][:, ns],
                                start=False, stop=False)
                for b in range(NB):
                    bs = slice(b * 128, (b + 1) * 128)
                    for h in range(NH):
                        ns = slice(h * 512, (h + 1) * 512)
                        nc.tensor.matmul(
                            psums[b, h][:], ones[:], bias2s[f][:, ns],
                            start=False, stop=True)
                        ot = obp.tile([128, 512], F32, tag="o", name=f"o_{f}_{b}_{h}")
                        nc.scalar.mul(ot[:], psums[b, h][:], 0.5)
                        nc.sync.dma_start(out_d[f, bs, ns], ot[:])

    _split_multi_waits(nc)
    return nc


_NC_CACHE = []


def kernel(**inputs) -> np.ndarray:
    x = np.asarray(inputs["history_in"], dtype=np.float32)     # [B, T, F]
    wt = np.asarray(inputs["trend_W"], dtype=np.float32)       # [F, T, N]
    wr = np.asarray(inputs["residual_W"], dtype=np.float32)    # [F, T, N]
    tb = np.asarray(inputs["trend_b"], dtype=np.float32)       # [F, N]
    rb = np.asarray(inputs["residual_b"], dtype=np.float32)    # [F, N]

    xT = x.transpose(2, 1, 0)                                  # [F, T, B] view
    # partition-major: xA[f, p, c, b] = xT[f, c*128+p, b]
    xA = np.ascontiguousarray(
        xT.reshape(F, TC, 128, B).transpose(0, 2, 1, 3))       # [F, 128, TC, B]
    # shifted-by-one-row copy with last row duplicated
    xTs = np.concatenate([xT[:, 1:, :], xT[:, T - 1:T, :]], axis=1)
    xB = np.ascontiguousarray(
        xTs.reshape(F, TC, 128, B).transpose(0, 2, 1, 3))      # [F, 128, TC, B]

    if not _NC_CACHE:
        _NC_CACHE.append(_build())
    nc = _NC_CACHE[0]

    in_maps = []
    for k in range(NCORES):
        sl = slice(FL * k, FL * (k + 1))
        in_maps.append({
            "xA": np.ascontiguousarray(xA[sl]),
            "xB": np.ascontiguousarray(xB[sl]),
            "Wt": np.ascontiguousarray(wt[sl]),
            "Wr": np.ascontiguousarray(wr[sl]),
            "tb": np.ascontiguousarray(tb[sl]),
            "rb": np.ascontiguousarray(rb[sl]),
            "ones": np.ones((1, 128), dtype=np.float32),
        })

    res = run_bass_kernel_spmd(nc, in_maps, core_ids=list(range(NCORES)))
    full = np.concatenate([r["out"] for r in res.results], axis=0)  # [F, B, N]
    return np.ascontiguousarray(full.transpose(1, 2, 0))            # [B, N, F]
